# revision 9
# baseline (speedup 1.0000x reference)
"""2-layer GCN (PyG GCNConv x2 + global mean pool + log_softmax) on 8 TRN2
NeuronCores via Bass/Tile.

Strategy (graph/data parallel, per the sharding hint):
  - Nodes are sharded graph-aligned: core c owns graphs [64c, 64c+64) and
    their nodes (~25k each). Each core owns all edges POINTING INTO its
    nodes, so aggregation is local; the only collective is one AllGather of
    the layer-2 propagate input (y2 = dinv * (h1 @ W2), N x 3).
  - Algebra: out = D^-1/2 (A+I) D^-1/2 x  ==>  y = dinv*x; agg[n] = sum over
    in-edges of y[src]; out = dinv * (agg + y). No per-edge norm needed.
  - Host prep is STRUCTURE-ONLY (int sort/bucketing of edge_index, slot
    layout, offset tables). All floating-point math runs on the device:
    dinv=1/sqrt(deg), y tables, per-edge gathers (indirect DMA), per-degree-
    class dense reductions, both matmuls, self-loop correction, graph
    pooling (PE one-hot matmuls), and log_softmax.
  - Per-edge gathers use [128,1]-offset indirect DMA instructions (one
    descriptor per partition), the only reliably-correct indirect form on
    this toolchain. Slots are laid out in a [128 x NCOLS] grid, nodes
    grouped by degree class so the segment reduction is dense.
"""

import os
import sys
import types

import numpy as np

N = 200_000
E = 3_200_000
G = 512
NCORES = 8
GPC = G // NCORES  # graphs per core
D_IN, D_HID, D_OUT = 2, 64, 3
N1 = 200_064  # 128 * 1563: natural-order table rows (padded)
NT_NAT = 3  # y1-table build tiles; 1563 = 3 * 521
FT_NAT = 521

TRACE = False  # set by test.py to capture an NTFF profile
TRACE_CORES = None
LAST_EXEC_NS = None
LAST_RESULTS = None


def _install_ntff_shim():
    """Allow bass_utils trace=True under axon (antenv.axon_hooks is absent
    in this image) and keep profile artifacts local."""
    if "antenv.axon_hooks" in sys.modules:
        return
    try:
        from trn_agent_boot.trn_boot import _ntff_profile_via_ctypes

        hook = _ntff_profile_via_ctypes("/opt/axon/libaxon_pjrt.so")
        import antenv

        mod = types.ModuleType("antenv.axon_hooks")
        mod.get_axon_ntff_profile_hook = lambda: hook
        mod.set_axon_ntff_profile_hook = lambda h: None
        sys.modules["antenv.axon_hooks"] = mod
        antenv.axon_hooks = mod
        import concourse.bass_utils as bu

        bu.upload_artifacts = lambda tmpdir: tmpdir
    except Exception:
        pass


def _ranges(lens):
    """[2,3] -> [0,1,0,1,2]"""
    tot = int(lens.sum())
    starts = np.repeat(np.cumsum(lens) - lens, lens)
    return np.arange(tot, dtype=np.int64) - starts


def _host_structure(src, dst, batch):
    """Structure-only preprocessing: slot grid + offset tables. Returns a
    dict of per-core numpy arrays and layout constants."""
    node_core = batch // GPC  # [N]
    deg = np.bincount(dst, minlength=N).astype(np.int64)

    order = np.argsort(dst, kind="stable")
    src_s = src[order]
    dst_s = dst[order]
    rowptr = np.zeros(N + 1, np.int64)
    np.cumsum(deg, out=rowptr[1:])

    # degree classes: ceil to multiple of 2, min 2
    K_of = np.maximum(((deg + 1) // 2) * 2, 2)
    classes = np.unique(K_of)
    cls_idx = np.searchsorted(classes, K_of)
    ncls = len(classes)

    grp = node_core * ncls + cls_idx
    cnt_cc = np.bincount(grp, minlength=NCORES * ncls).reshape(NCORES, ncls)
    m_cls = ((cnt_cc.max(axis=0) + 127) // 128) * 128
    R_cls = (m_cls // 128).astype(np.int64)
    # pad NPP to a multiple of 16 (so 128*NPP % 2048 == 0) with dummy rows
    # in the smallest class
    R_cls[0] += (-int(R_cls.sum())) % 16
    NPP = int(R_cls.sum())
    C0 = np.zeros(ncls, np.int64)
    A0 = np.zeros(ncls, np.int64)
    C0[1:] = np.cumsum(R_cls[:-1] * classes[:-1])
    A0[1:] = np.cumsum(R_cls[:-1])
    NCOLS = int((R_cls * classes).sum())
    NPG = 128 * NPP

    # rank of each node within its (core, class) group
    order_n = np.lexsort((np.arange(N), grp))
    rank = np.empty(N, np.int64)
    grp_sorted = grp[order_n]
    gstart = np.searchsorted(grp_sorted, np.arange(NCORES * ncls))
    rank[order_n] = np.arange(N) - gstart[grp_sorted]

    p_of = rank % 128
    r_of = rank // 128
    K_arr = classes[cls_idx]
    col0_of = C0[cls_idx] + r_of * K_arr
    jj_of = A0[cls_idx] + r_of
    flat_of = p_of * NPP + jj_of  # per-core flat grid index
    gflat_of = node_core * NPG + flat_of  # global row in AllGather table

    # slot -> default offsets (pad slots point at the node's own row; the
    # device subtracts padcnt * y_own, supplied as pm1 = 1 - padcnt)
    base = node_core * (128 * NCOLS) + p_of * NCOLS + col0_of
    tot_slots = NCORES * 128 * NCOLS
    offs1 = np.zeros(tot_slots, np.int32)
    offs2 = np.zeros(tot_slots, np.int32)
    node_rep = np.repeat(np.arange(N, dtype=np.int64), K_arr)
    slot_rep = np.repeat(base, K_arr) + _ranges(K_arr)
    offs1[slot_rep] = node_rep.astype(np.int32)
    offs2[slot_rep] = gflat_of[node_rep].astype(np.int32)
    # real edges overwrite their slots
    slot_e = base[dst_s] + (np.arange(E, dtype=np.int64) - rowptr[dst_s])
    offs1[slot_e] = src_s.astype(np.int32)
    offs2[slot_e] = gflat_of[src_s].astype(np.int32)
    offs1 = offs1.reshape(NCORES, 128, NCOLS)
    offs2 = offs2.reshape(NCORES, 128, NCOLS)

    # per-node grid arrays [NCORES, 128, NPP]
    deg_g = np.ones((NCORES * NPG,), np.float32)
    deg_g[gflat_of] = (deg + 1).astype(np.float32)
    pm1_g = np.ones((NCORES * NPG,), np.float32)
    pm1_g[gflat_of] = (1.0 - (K_arr - deg)).astype(np.float32)
    gidx_g = np.full((NCORES * NPG,), -1.0, np.float32)
    gidx_g[gflat_of] = (batch - node_core * GPC).astype(np.float32)

    cnt = np.bincount(batch, minlength=G).astype(np.float32)
    cnt = np.maximum(cnt, 1.0).reshape(NCORES, GPC)

    class_blocks = [
        (int(C0[i]), int(A0[i]), int(R_cls[i]), int(classes[i]))
        for i in range(ncls)
    ]
    return dict(
        NCOLS=NCOLS,
        NPP=NPP,
        NPG=NPG,
        class_blocks=class_blocks,
        offs1=offs1,
        offs2=offs2,
        deg_g=deg_g.reshape(NCORES, 128, NPP),
        pm1_g=pm1_g.reshape(NCORES, 128, NPP),
        gidx_g=gidx_g.reshape(NCORES, 128, NPP),
        cnt=cnt,
        gflat_of=gflat_of,
        deg=deg,
    )


def _build_program(NCOLS, NPP, class_blocks):
    from concourse import bass, bacc, mybir
    import concourse.tile as tile

    dt = mybir.dt
    f32 = dt.float32
    Alu = mybir.AluOpType
    Act = mybir.ActivationFunctionType
    Ax = mybir.AxisListType
    NPG = 128 * NPP

    nc = bacc.Bacc("TRN2", target_bir_lowering=False, debug=False,
                   num_devices=NCORES)
    x_nat = nc.dram_tensor("x_nat", [N1, 2], f32, kind="ExternalInput")
    deg_nat = nc.dram_tensor("deg_nat", [N1, 1], f32, kind="ExternalInput")
    offs1_d = nc.dram_tensor("offs1", [128, NCOLS], dt.int32, kind="ExternalInput")
    offs2_d = nc.dram_tensor("offs2", [128, NCOLS], dt.int32, kind="ExternalInput")
    degg_d = nc.dram_tensor("deg_g", [128, NPP], f32, kind="ExternalInput")
    pm1_d = nc.dram_tensor("pm1_g", [128, NPP], f32, kind="ExternalInput")
    gidx_d = nc.dram_tensor("gidx_g", [128, NPP], f32, kind="ExternalInput")
    xg_d = nc.dram_tensor("x_g", [128, NPP * 2], f32, kind="ExternalInput")
    W1_d = nc.dram_tensor("W1", [2, 64], f32, kind="ExternalInput")
    b1_d = nc.dram_tensor("b1c", [64, 1], f32, kind="ExternalInput")
    W2_d = nc.dram_tensor("W2", [64, 3], f32, kind="ExternalInput")
    b2_d = nc.dram_tensor("b2r", [128, 3], f32, kind="ExternalInput")
    iota_d = nc.dram_tensor("iota64", [128, 64], f32, kind="ExternalInput")
    cnt_d = nc.dram_tensor("cnt", [64, 1], f32, kind="ExternalInput")
    out_d = nc.dram_tensor("out", [64, 3], f32, kind="ExternalOutput")

    with tile.TileContext(nc) as tc:
        with (
            tc.tile_pool(name="sb", bufs=1) as sb,
            tc.tile_pool(name="wk", bufs=1) as wk,
            tc.tile_pool(name="ps", bufs=2, space="PSUM") as ps,
            tc.tile_pool(name="dr", bufs=1, space="DRAM") as dr,
        ):
            # ---------- y1 table: y1[n] = x[n] / sqrt(deg_ref[n]) ----------
            y1tab = dr.tile([N1, 2], f32)
            x_view = x_nat.ap().rearrange("(t p f) c -> t p (f c)", t=NT_NAT, p=128)
            d_view = deg_nat.ap().rearrange("(t p f) c -> t p (f c)", t=NT_NAT, p=128)
            y_view = y1tab[:].rearrange("(t p f) c -> t p (f c)", t=NT_NAT, p=128)
            for t in range(NT_NAT):
                xt = wk.tile([128, FT_NAT * 2], f32, tag="xt")
                dg = wk.tile([128, FT_NAT], f32, tag="dgn")
                nc.sync.dma_start(out=xt[:], in_=x_view[t])
                nc.sync.dma_start(out=dg[:], in_=d_view[t])
                sq = wk.tile([128, FT_NAT], f32, tag="sqn")
                nc.scalar.activation(sq[:], dg[:], Act.Sqrt)
                dv = wk.tile([128, FT_NAT], f32, tag="dvn")
                nc.vector.reciprocal(dv[:], sq[:])
                y1t = wk.tile([128, FT_NAT * 2], f32, tag="y1t")
                nc.vector.tensor_tensor(
                    out=y1t[:].rearrange("p (f c) -> p f c", c=2),
                    in0=xt[:].rearrange("p (f c) -> p f c", c=2),
                    in1=dv[:].unsqueeze(2).to_broadcast([128, FT_NAT, 2]),
                    op=Alu.mult,
                )
                nc.sync.dma_start(out=y_view[t], in_=y1t[:])

            # ---------- persistent loads ----------
            offs1_t = sb.tile([128, NCOLS], dt.int32)
            nc.sync.dma_start(out=offs1_t[:], in_=offs1_d.ap())
            offs2_t = sb.tile([128, NCOLS], dt.int32)
            nc.sync.dma_start(out=offs2_t[:], in_=offs2_d.ap())
            degg = sb.tile([128, NPP], f32)
            nc.sync.dma_start(out=degg[:], in_=degg_d.ap())
            pm1 = sb.tile([128, NPP], f32)
            nc.sync.dma_start(out=pm1[:], in_=pm1_d.ap())
            xg = sb.tile([128, NPP * 2], f32)
            nc.sync.dma_start(out=xg[:], in_=xg_d.ap())
            W1sb = sb.tile([2, 64], f32)
            nc.sync.dma_start(out=W1sb[:], in_=W1_d.ap())
            b1sb = sb.tile([64, 1], f32)
            nc.sync.dma_start(out=b1sb[:], in_=b1_d.ap())
            W2sb = sb.tile([64, 3], f32)
            nc.sync.dma_start(out=W2sb[:], in_=W2_d.ap())
            b2sb = sb.tile([128, 3], f32)
            nc.sync.dma_start(out=b2sb[:], in_=b2_d.ap())

            # dinv over the grid
            sqg = wk.tile([128, NPP], f32, tag="sqg")
            nc.scalar.activation(sqg[:], degg[:], Act.Sqrt)
            dvg = sb.tile([128, NPP], f32)
            nc.vector.reciprocal(dvg[:], sqg[:])

            def bcast(ap_2d, d):
                return ap_2d.unsqueeze(2).to_broadcast([128, NPP, d])

            def as3(ap_1d, d):
                return ap_1d.rearrange("p (j c) -> p j c", c=d)

            # y1own = x_g * dinv (self-loop term)
            y1own = sb.tile([128, NPP * 2], f32)
            nc.vector.tensor_tensor(out=as3(y1own[:], 2), in0=as3(xg[:], 2),
                                    in1=bcast(dvg[:], 2), op=Alu.mult)

            # ---------- L1 gather ----------
            g1 = sb.tile([128, NCOLS * 2], f32, tag="big")  # shared with g2
            for i in range(NCOLS):
                nc.gpsimd.indirect_dma_start(
                    out=g1[:, 2 * i:2 * i + 2],
                    out_offset=None,
                    in_=y1tab[:],
                    in_offset=bass.IndirectOffsetOnAxis(
                        ap=offs1_t[:, i:i + 1], axis=0),
                )

            # ---------- L1 class reductions ----------
            agg1 = sb.tile([128, NPP * 2], f32)
            for (c0, a0, R, K) in class_blocks:
                nc.vector.tensor_reduce(
                    out=agg1[:, a0 * 2:(a0 + R) * 2].rearrange(
                        "p (r c) -> p r c", c=2),
                    in_=g1[:, c0 * 2:(c0 + R * K) * 2].rearrange(
                        "p (r k c) -> p r c k", r=R, k=K),
                    axis=Ax.X,
                    op=Alu.add,
                )

            # out1 = (agg1 + pm1 * y1own) * dinv
            t1 = wk.tile([128, NPP * 2], f32, tag="t1")
            nc.vector.tensor_tensor(out=as3(t1[:], 2), in0=as3(y1own[:], 2),
                                    in1=bcast(pm1[:], 2), op=Alu.mult)
            t2 = wk.tile([128, NPP * 2], f32, tag="t2")
            nc.vector.tensor_tensor(out=as3(t2[:], 2), in0=as3(agg1[:], 2),
                                    in1=as3(t1[:], 2), op=Alu.add)
            out1 = sb.tile([128, NPP * 2], f32)
            nc.vector.tensor_tensor(out=as3(out1[:], 2), in0=as3(t2[:], 2),
                                    in1=bcast(dvg[:], 2), op=Alu.mult)

            # ---------- bounce to flat layout for the MLP ----------
            o1f = dr.tile([NPG, 2], f32)
            nc.sync.dma_start(
                out=o1f[:].rearrange("(p j) c -> p (j c)", p=128),
                in_=out1[:])
            dvf = dr.tile([NPG, 1], f32)
            nc.sync.dma_start(
                out=dvf[:].rearrange("(p j) c -> p (j c)", p=128),
                in_=dvg[:])

            # ---------- h1 = relu(out1 @ W1 + b1); y2 = dinv*(h1 @ W2) ----
            # chunked through DRAM to bound SBUF usage
            y2loc = dr.tile([NPG, 3], f32)
            CH = 2048
            assert NPG % CH == 0
            for ic in range(NPG // CH):
                s0 = ic * CH
                o1T = wk.tile([2, CH], f32, tag="o1T")
                nc.sync.dma_start(
                    out=o1T[:],
                    in_=o1f[s0:s0 + CH, :].rearrange("r c -> c r"))
                dv3 = wk.tile([3, CH], f32, tag="dv3")
                for c in range(3):
                    nc.sync.dma_start(
                        out=dv3[c:c + 1, :],
                        in_=dvf[s0:s0 + CH, :].rearrange("r one -> one r"))
                y2t = wk.tile([3, CH], f32, tag="y2t")
                for it in range(CH // 512):
                    s = it * 512
                    p1 = ps.tile([64, 512], f32, tag="p1")
                    nc.tensor.matmul(out=p1[:], lhsT=W1sb[:],
                                     rhs=o1T[:, s:s + 512],
                                     start=True, stop=True)
                    h1 = wk.tile([64, 512], f32, tag="h1")
                    nc.scalar.activation(h1[:], p1[:], Act.Relu,
                                         bias=b1sb[:])
                    p2 = ps.tile([3, 512], f32, tag="p2")
                    nc.tensor.matmul(out=p2[:], lhsT=W2sb[:],
                                     rhs=h1[:], start=True, stop=True)
                    nc.vector.tensor_tensor(out=y2t[:, s:s + 512],
                                            in0=p2[:],
                                            in1=dv3[:, s:s + 512],
                                            op=Alu.mult)
                nc.sync.dma_start(
                    out=y2loc[s0:s0 + CH, :].rearrange("r c -> c r"),
                    in_=y2t[:])

            # ---------- AllGather y2 ----------
            y2tab = dr.tile([NCORES * NPG, 3], f32)
            nc.gpsimd.collective_compute(
                "AllGather",
                Alu.bypass,
                replica_groups=[list(range(NCORES))],
                ins=[y2loc.opt()],
                outs=[y2tab.opt()],
            )

            # y2 of own nodes, grid layout
            y2og = sb.tile([128, NPP * 3], f32)
            nc.sync.dma_start(
                out=y2og[:],
                in_=y2loc[:].rearrange("(p j) c -> p (j c)", p=128))

            # ---------- L2 gather ----------
            g2 = sb.tile([128, NCOLS * 3], f32)
            for i in range(NCOLS):
                nc.gpsimd.indirect_dma_start(
                    out=g2[:, 3 * i:3 * i + 3],
                    out_offset=None,
                    in_=y2tab[:],
                    in_offset=bass.IndirectOffsetOnAxis(
                        ap=offs2_t[:, i:i + 1], axis=0),
                )

            # ---------- L2 class reductions ----------
            agg2 = sb.tile([128, NPP * 3], f32)
            for (c0, a0, R, K) in class_blocks:
                nc.vector.tensor_reduce(
                    out=agg2[:, a0 * 3:(a0 + R) * 3].rearrange(
                        "p (r c) -> p r c", c=3),
                    in_=g2[:, c0 * 3:(c0 + R * K) * 3].rearrange(
                        "p (r k c) -> p r c k", r=R, k=K),
                    axis=Ax.X,
                    op=Alu.add,
                )

            # out2 = (agg2 + pm1 * y2own) * dinv + b2
            u1 = wk.tile([128, NPP * 3], f32, tag="u1")
            nc.vector.tensor_tensor(out=as3(u1[:], 3), in0=as3(y2og[:], 3),
                                    in1=bcast(pm1[:], 3), op=Alu.mult)
            u2 = wk.tile([128, NPP * 3], f32, tag="u2")
            nc.vector.tensor_tensor(out=as3(u2[:], 3), in0=as3(agg2[:], 3),
                                    in1=as3(u1[:], 3), op=Alu.add)
            u3 = wk.tile([128, NPP * 3], f32, tag="u3")
            nc.vector.tensor_tensor(out=as3(u3[:], 3), in0=as3(u2[:], 3),
                                    in1=bcast(dvg[:], 3), op=Alu.mult)
            out2 = sb.tile([128, NPP * 3], f32)
            nc.vector.tensor_tensor(
                out=as3(out2[:], 3), in0=as3(u3[:], 3),
                in1=b2sb[:].unsqueeze(1).to_broadcast([128, NPP, 3]),
                op=Alu.add)

            # ---------- pooling: one-hot over 64 graphs, PE accumulate ----
            gidx = sb.tile([128, NPP], f32)
            nc.sync.dma_start(out=gidx[:], in_=gidx_d.ap())
            iota = sb.tile([128, 64], f32)
            nc.sync.dma_start(out=iota[:], in_=iota_d.ap())
            ppool = ps.tile([64, 3], f32, tag="pp")
            OHC = 8
            assert NPP % OHC == 0
            for jc in range(NPP // OHC):
                j0 = jc * OHC
                oh = wk.tile([128, OHC * 64], f32, tag="oh")
                nc.vector.tensor_tensor(
                    out=oh[:].rearrange("p (j g) -> p j g", g=64),
                    in0=gidx[:, j0:j0 + OHC].unsqueeze(2).to_broadcast(
                        [128, OHC, 64]),
                    in1=iota[:].unsqueeze(1).to_broadcast([128, OHC, 64]),
                    op=Alu.is_equal,
                )
                for j in range(OHC):
                    jj = j0 + j
                    nc.tensor.matmul(out=ppool[:],
                                     lhsT=oh[:, 64 * j:64 * (j + 1)],
                                     rhs=out2[:, 3 * jj:3 * jj + 3],
                                     start=(jj == 0), stop=(jj == NPP - 1))

            cntsb = wk.tile([64, 1], f32, tag="cnt")
            nc.sync.dma_start(out=cntsb[:], in_=cnt_d.ap())
            rcnt = wk.tile([64, 1], f32, tag="rcnt")
            nc.vector.reciprocal(rcnt[:], cntsb[:])
            pooled = wk.tile([64, 3], f32, tag="pool")
            nc.vector.tensor_tensor(out=pooled[:], in0=ppool[:],
                                    in1=rcnt[:].to_broadcast([64, 3]),
                                    op=Alu.mult)
            # log_softmax
            mx = wk.tile([64, 1], f32, tag="mx")
            nc.vector.tensor_reduce(out=mx[:], in_=pooled[:], axis=Ax.X,
                                    op=Alu.max)
            z = wk.tile([64, 3], f32, tag="z")
            nc.vector.tensor_tensor(out=z[:], in0=pooled[:],
                                    in1=mx[:].to_broadcast([64, 3]),
                                    op=Alu.subtract)
            ez = wk.tile([64, 3], f32, tag="ez")
            nc.scalar.activation(ez[:], z[:], Act.Exp)
            sz = wk.tile([64, 1], f32, tag="sz")
            nc.vector.tensor_reduce(out=sz[:], in_=ez[:], axis=Ax.X,
                                    op=Alu.add)
            lz = wk.tile([64, 1], f32, tag="lz")
            nc.scalar.activation(lz[:], sz[:], Act.Ln)
            outz = wk.tile([64, 3], f32, tag="outz")
            nc.vector.tensor_tensor(out=outz[:], in0=z[:],
                                    in1=lz[:].to_broadcast([64, 3]),
                                    op=Alu.subtract)
            nc.sync.dma_start(out=out_d.ap(), in_=outz[:])

    nc.finalize()
    return nc


def kernel(x, edge_index, batch, W1, b1, W2, b2):
    global LAST_EXEC_NS, LAST_RESULTS
    if TRACE:
        _install_ntff_shim()
    from concourse.bass_utils import run_bass_kernel_spmd

    x = np.ascontiguousarray(np.asarray(x, dtype=np.float32))
    src = np.asarray(edge_index[0]).astype(np.int64)
    dst = np.asarray(edge_index[1]).astype(np.int64)
    batch = np.asarray(batch).astype(np.int64)
    W1 = np.asarray(W1, dtype=np.float32)
    b1 = np.asarray(b1, dtype=np.float32)
    W2 = np.asarray(W2, dtype=np.float32)
    b2 = np.asarray(b2, dtype=np.float32)

    st = _host_structure(src, dst, batch)
    NCOLS, NPP, NPG = st["NCOLS"], st["NPP"], st["NPG"]

    # natural-order tables (replicated to all cores)
    x_nat = np.zeros((N1, 2), np.float32)
    x_nat[:N] = x
    deg_nat = np.ones((N1, 1), np.float32)
    deg_nat[:N, 0] = (st["deg"] + 1).astype(np.float32)

    # x in grid order (pure row movement; zero for dummy rows)
    x_g = np.zeros((NCORES * NPG, 2), np.float32)
    x_g[st["gflat_of"]] = x
    x_g = x_g.reshape(NCORES, 128, NPP * 2)

    iota64 = np.broadcast_to(np.arange(64, dtype=np.float32), (128, 64)).copy()

    nc = _build_program(NCOLS, NPP, st["class_blocks"])

    in_maps = []
    for c in range(NCORES):
        in_maps.append({
            "x_nat": x_nat,
            "deg_nat": deg_nat,
            "offs1": st["offs1"][c],
            "offs2": st["offs2"][c],
            "deg_g": st["deg_g"][c],
            "pm1_g": st["pm1_g"][c],
            "gidx_g": st["gidx_g"][c],
            "x_g": x_g[c],
            "W1": W1,
            "b1c": b1.reshape(64, 1),
            "W2": W2,
            "b2r": np.broadcast_to(b2, (128, 3)).copy(),
            "iota64": iota64,
            "cnt": st["cnt"][c].reshape(GPC, 1),
        })

    kwargs = {}
    if TRACE:
        kwargs = dict(trace=True,
                      trace_cores=TRACE_CORES or list(range(NCORES)))
    res = run_bass_kernel_spmd(nc, in_maps, core_ids=list(range(NCORES)),
                               **kwargs)
    LAST_EXEC_NS = res.exec_time_ns
    LAST_RESULTS = res
    out = np.concatenate([res.results[c]["out"] for c in range(NCORES)], axis=0)
    return out.astype(np.float32)


# revision 10
# speedup vs baseline: 1.0067x; 1.0067x over previous
"""2-layer GCN (PyG GCNConv x2 + global mean pool + log_softmax) on 8 TRN2
NeuronCores via Bass/Tile.

Strategy (graph/data parallel, per the sharding hint):
  - Nodes are sharded graph-aligned: core c owns graphs [64c, 64c+64) and
    their nodes (~25k each). Each core owns all edges POINTING INTO its
    nodes, so aggregation is local; the only collective is one AllGather of
    the layer-2 propagate input (y2 = dinv * (h1 @ W2), N x 3).
  - Algebra: out = D^-1/2 (A+I) D^-1/2 x  ==>  y = dinv*x; agg[n] = sum over
    in-edges of y[src]; out = dinv * (agg + y). No per-edge norm needed.
  - Host prep is STRUCTURE-ONLY (int sort/bucketing of edge_index, slot
    layout, offset tables). All floating-point math runs on the device:
    dinv=1/sqrt(deg), y tables, per-edge gathers (indirect DMA), per-degree-
    class dense reductions, both matmuls, self-loop correction, graph
    pooling (PE one-hot matmuls), and log_softmax.
  - Per-edge gathers use [128,1]-offset indirect DMA instructions (one
    descriptor per partition), the only reliably-correct indirect form on
    this toolchain. Slots are laid out in a [128 x NCOLS] grid, nodes
    grouped by degree class so the segment reduction is dense.
"""

import os
import sys
import types

import numpy as np

N = 200_000
E = 3_200_000
G = 512
NCORES = 8
GPC = G // NCORES  # graphs per core
D_IN, D_HID, D_OUT = 2, 64, 3
N1 = 200_064  # 128 * 1563: natural-order table rows (padded)
NT_NAT = 3  # y1-table build tiles; 1563 = 3 * 521
FT_NAT = 521

TRACE = False  # set by test.py to capture an NTFF profile
TRACE_CORES = None
LAST_EXEC_NS = None
LAST_RESULTS = None


def _install_ntff_shim():
    """Allow bass_utils trace=True under axon (antenv.axon_hooks is absent
    in this image) and keep profile artifacts local."""
    if "antenv.axon_hooks" in sys.modules:
        return
    try:
        from trn_agent_boot.trn_boot import _ntff_profile_via_ctypes

        hook = _ntff_profile_via_ctypes("/opt/axon/libaxon_pjrt.so")
        import antenv

        mod = types.ModuleType("antenv.axon_hooks")
        mod.get_axon_ntff_profile_hook = lambda: hook
        mod.set_axon_ntff_profile_hook = lambda h: None
        sys.modules["antenv.axon_hooks"] = mod
        antenv.axon_hooks = mod
        import concourse.bass_utils as bu

        bu.upload_artifacts = lambda tmpdir: tmpdir
    except Exception:
        pass


def _ranges(lens):
    """[2,3] -> [0,1,0,1,2]"""
    tot = int(lens.sum())
    starts = np.repeat(np.cumsum(lens) - lens, lens)
    return np.arange(tot, dtype=np.int64) - starts


def _host_structure(src, dst, batch):
    """Structure-only preprocessing: slot grid + offset tables. Returns a
    dict of per-core numpy arrays and layout constants."""
    node_core = batch // GPC  # [N]
    deg = np.bincount(dst, minlength=N).astype(np.int64)

    order = np.argsort(dst, kind="stable")
    src_s = src[order]
    dst_s = dst[order]
    rowptr = np.zeros(N + 1, np.int64)
    np.cumsum(deg, out=rowptr[1:])

    # degree classes: ceil to multiple of 2, min 2
    K_of = np.maximum(((deg + 1) // 2) * 2, 2)
    classes = np.unique(K_of)
    cls_idx = np.searchsorted(classes, K_of)
    ncls = len(classes)

    grp = node_core * ncls + cls_idx
    cnt_cc = np.bincount(grp, minlength=NCORES * ncls).reshape(NCORES, ncls)
    m_cls = ((cnt_cc.max(axis=0) + 127) // 128) * 128
    R_cls = (m_cls // 128).astype(np.int64)
    # pad NPP to a multiple of 16 (so 128*NPP % 2048 == 0) with dummy rows
    # in the smallest class
    R_cls[0] += (-int(R_cls.sum())) % 16
    NPP = int(R_cls.sum())
    C0 = np.zeros(ncls, np.int64)
    A0 = np.zeros(ncls, np.int64)
    C0[1:] = np.cumsum(R_cls[:-1] * classes[:-1])
    A0[1:] = np.cumsum(R_cls[:-1])
    NCOLS = int((R_cls * classes).sum())
    NPG = 128 * NPP

    # rank of each node within its (core, class) group
    order_n = np.lexsort((np.arange(N), grp))
    rank = np.empty(N, np.int64)
    grp_sorted = grp[order_n]
    gstart = np.searchsorted(grp_sorted, np.arange(NCORES * ncls))
    rank[order_n] = np.arange(N) - gstart[grp_sorted]

    p_of = rank % 128
    r_of = rank // 128
    K_arr = classes[cls_idx]
    col0_of = C0[cls_idx] + r_of * K_arr
    jj_of = A0[cls_idx] + r_of
    flat_of = p_of * NPP + jj_of  # per-core flat grid index
    gflat_of = node_core * NPG + flat_of  # global row in AllGather table

    # slot -> default offsets (pad slots point at the node's own row; the
    # device subtracts padcnt * y_own, supplied as pm1 = 1 - padcnt)
    base = node_core * (128 * NCOLS) + p_of * NCOLS + col0_of
    tot_slots = NCORES * 128 * NCOLS
    offs1 = np.zeros(tot_slots, np.int32)
    offs2 = np.zeros(tot_slots, np.int32)
    node_rep = np.repeat(np.arange(N, dtype=np.int64), K_arr)
    slot_rep = np.repeat(base, K_arr) + _ranges(K_arr)
    offs1[slot_rep] = node_rep.astype(np.int32)
    offs2[slot_rep] = gflat_of[node_rep].astype(np.int32)
    # real edges overwrite their slots
    slot_e = base[dst_s] + (np.arange(E, dtype=np.int64) - rowptr[dst_s])
    offs1[slot_e] = src_s.astype(np.int32)
    offs2[slot_e] = gflat_of[src_s].astype(np.int32)
    offs1 = offs1.reshape(NCORES, 128, NCOLS)
    offs2 = offs2.reshape(NCORES, 128, NCOLS)

    # per-node grid arrays [NCORES, 128, NPP]
    deg_g = np.ones((NCORES * NPG,), np.float32)
    deg_g[gflat_of] = (deg + 1).astype(np.float32)
    pm1_g = np.ones((NCORES * NPG,), np.float32)
    pm1_g[gflat_of] = (1.0 - (K_arr - deg)).astype(np.float32)
    gidx_g = np.full((NCORES * NPG,), -1.0, np.float32)
    gidx_g[gflat_of] = (batch - node_core * GPC).astype(np.float32)

    cnt = np.bincount(batch, minlength=G).astype(np.float32)
    cnt = np.maximum(cnt, 1.0).reshape(NCORES, GPC)

    class_blocks = [
        (int(C0[i]), int(A0[i]), int(R_cls[i]), int(classes[i]))
        for i in range(ncls)
    ]
    return dict(
        NCOLS=NCOLS,
        NPP=NPP,
        NPG=NPG,
        class_blocks=class_blocks,
        offs1=offs1,
        offs2=offs2,
        deg_g=deg_g.reshape(NCORES, 128, NPP),
        pm1_g=pm1_g.reshape(NCORES, 128, NPP),
        gidx_g=gidx_g.reshape(NCORES, 128, NPP),
        cnt=cnt,
        gflat_of=gflat_of,
        deg=deg,
    )


def _build_program(NCOLS, NPP, class_blocks):
    from concourse import bass, bacc, mybir
    import concourse.tile as tile

    dt = mybir.dt
    f32 = dt.float32
    Alu = mybir.AluOpType
    Act = mybir.ActivationFunctionType
    Ax = mybir.AxisListType
    NPG = 128 * NPP

    nc = bacc.Bacc("TRN2", target_bir_lowering=False, debug=False,
                   num_devices=NCORES)
    x_nat = nc.dram_tensor("x_nat", [N1, 2], f32, kind="ExternalInput")
    deg_nat = nc.dram_tensor("deg_nat", [N1, 1], f32, kind="ExternalInput")
    offs1_d = nc.dram_tensor("offs1", [128, NCOLS], dt.int32, kind="ExternalInput")
    offs2_d = nc.dram_tensor("offs2", [128, NCOLS], dt.int32, kind="ExternalInput")
    degg_d = nc.dram_tensor("deg_g", [128, NPP], f32, kind="ExternalInput")
    pm1_d = nc.dram_tensor("pm1_g", [128, NPP], f32, kind="ExternalInput")
    gidx_d = nc.dram_tensor("gidx_g", [128, NPP], f32, kind="ExternalInput")
    xg_d = nc.dram_tensor("x_g", [128, NPP * 2], f32, kind="ExternalInput")
    W1_d = nc.dram_tensor("W1", [2, 64], f32, kind="ExternalInput")
    b1_d = nc.dram_tensor("b1c", [64, 1], f32, kind="ExternalInput")
    W2_d = nc.dram_tensor("W2", [64, 3], f32, kind="ExternalInput")
    b2_d = nc.dram_tensor("b2r", [128, 3], f32, kind="ExternalInput")
    iota_d = nc.dram_tensor("iota64", [128, 64], f32, kind="ExternalInput")
    cnt_d = nc.dram_tensor("cnt", [64, 1], f32, kind="ExternalInput")
    out_d = nc.dram_tensor("out", [64, 3], f32, kind="ExternalOutput")

    with tile.TileContext(nc) as tc:
        with (
            tc.tile_pool(name="sb", bufs=1) as sb,
            tc.tile_pool(name="wk", bufs=1) as wk,
            tc.tile_pool(name="ps", bufs=2, space="PSUM") as ps,
            tc.tile_pool(name="dr", bufs=1, space="DRAM") as dr,
        ):
            # ---------- y1 table: y1[n] = x[n] / sqrt(deg_ref[n]) ----------
            y1tab = dr.tile([N1, 2], f32)
            x_view = x_nat.ap().rearrange("(t p f) c -> t p (f c)", t=NT_NAT, p=128)
            d_view = deg_nat.ap().rearrange("(t p f) c -> t p (f c)", t=NT_NAT, p=128)
            y_view = y1tab[:].rearrange("(t p f) c -> t p (f c)", t=NT_NAT, p=128)
            for t in range(NT_NAT):
                xt = wk.tile([128, FT_NAT * 2], f32, tag="xt")
                dg = wk.tile([128, FT_NAT], f32, tag="dgn")
                nc.sync.dma_start(out=xt[:], in_=x_view[t])
                nc.sync.dma_start(out=dg[:], in_=d_view[t])
                sq = wk.tile([128, FT_NAT], f32, tag="sqn")
                nc.scalar.activation(sq[:], dg[:], Act.Sqrt)
                dv = wk.tile([128, FT_NAT], f32, tag="dvn")
                nc.vector.reciprocal(dv[:], sq[:])
                y1t = wk.tile([128, FT_NAT * 2], f32, tag="y1t")
                nc.vector.tensor_tensor(
                    out=y1t[:].rearrange("p (f c) -> p f c", c=2),
                    in0=xt[:].rearrange("p (f c) -> p f c", c=2),
                    in1=dv[:].unsqueeze(2).to_broadcast([128, FT_NAT, 2]),
                    op=Alu.mult,
                )
                nc.sync.dma_start(out=y_view[t], in_=y1t[:])

            # ---------- persistent loads ----------
            offs1_t = sb.tile([128, NCOLS], dt.int32)
            nc.sync.dma_start(out=offs1_t[:], in_=offs1_d.ap())
            offs2_t = sb.tile([128, NCOLS], dt.int32)
            nc.sync.dma_start(out=offs2_t[:], in_=offs2_d.ap())
            degg = sb.tile([128, NPP], f32)
            nc.sync.dma_start(out=degg[:], in_=degg_d.ap())
            pm1 = sb.tile([128, NPP], f32)
            nc.sync.dma_start(out=pm1[:], in_=pm1_d.ap())
            xg = sb.tile([128, NPP * 2], f32)
            nc.sync.dma_start(out=xg[:], in_=xg_d.ap())
            W1sb = sb.tile([2, 64], f32)
            nc.sync.dma_start(out=W1sb[:], in_=W1_d.ap())
            b1sb = sb.tile([64, 1], f32)
            nc.sync.dma_start(out=b1sb[:], in_=b1_d.ap())
            W2sb = sb.tile([64, 3], f32)
            nc.sync.dma_start(out=W2sb[:], in_=W2_d.ap())
            b2sb = sb.tile([128, 3], f32)
            nc.sync.dma_start(out=b2sb[:], in_=b2_d.ap())

            # dinv over the grid
            sqg = wk.tile([128, NPP], f32, tag="sqg")
            nc.scalar.activation(sqg[:], degg[:], Act.Sqrt)
            dvg = sb.tile([128, NPP], f32)
            nc.vector.reciprocal(dvg[:], sqg[:])

            def bcast(ap_2d, d):
                return ap_2d.unsqueeze(2).to_broadcast([128, NPP, d])

            def as3(ap_1d, d):
                return ap_1d.rearrange("p (j c) -> p j c", c=d)

            # y1own = x_g * dinv (self-loop term)
            y1own = sb.tile([128, NPP * 2], f32)
            nc.vector.tensor_tensor(out=as3(y1own[:], 2), in0=as3(xg[:], 2),
                                    in1=bcast(dvg[:], 2), op=Alu.mult)

            # ---------- L1 gather ----------
            g1 = sb.tile([128, NCOLS * 2], f32, tag="big")  # shared with g2
            for i in range(NCOLS):
                nc.gpsimd.indirect_dma_start(
                    out=g1[:, 2 * i:2 * i + 2],
                    out_offset=None,
                    in_=y1tab[:],
                    in_offset=bass.IndirectOffsetOnAxis(
                        ap=offs1_t[:, i:i + 1], axis=0),
                )

            # ---------- L1 class reductions ----------
            agg1 = sb.tile([128, NPP * 2], f32)
            for (c0, a0, R, K) in class_blocks:
                nc.vector.tensor_reduce(
                    out=agg1[:, a0 * 2:(a0 + R) * 2].rearrange(
                        "p (r c) -> p r c", c=2),
                    in_=g1[:, c0 * 2:(c0 + R * K) * 2].rearrange(
                        "p (r k c) -> p r c k", r=R, k=K),
                    axis=Ax.X,
                    op=Alu.add,
                )

            # out1 = (agg1 + pm1 * y1own) * dinv
            t1 = wk.tile([128, NPP * 2], f32, tag="t1")
            nc.vector.tensor_tensor(out=as3(t1[:], 2), in0=as3(y1own[:], 2),
                                    in1=bcast(pm1[:], 2), op=Alu.mult)
            t2 = wk.tile([128, NPP * 2], f32, tag="t2")
            nc.vector.tensor_tensor(out=as3(t2[:], 2), in0=as3(agg1[:], 2),
                                    in1=as3(t1[:], 2), op=Alu.add)
            out1 = sb.tile([128, NPP * 2], f32)
            nc.vector.tensor_tensor(out=as3(out1[:], 2), in0=as3(t2[:], 2),
                                    in1=bcast(dvg[:], 2), op=Alu.mult)

            # ---------- bounce to flat layout for the MLP ----------
            o1f = dr.tile([NPG, 2], f32)
            nc.sync.dma_start(
                out=o1f[:].rearrange("(p j) c -> p (j c)", p=128),
                in_=out1[:])
            dvf = dr.tile([NPG, 1], f32)
            nc.sync.dma_start(
                out=dvf[:].rearrange("(p j) c -> p (j c)", p=128),
                in_=dvg[:])

            # ---------- h1 = relu(out1 @ W1 + b1); y2 = dinv*(h1 @ W2) ----
            # chunked through DRAM to bound SBUF usage
            y2loc = dr.tile([NPG, 3], f32)
            CH = 2048
            assert NPG % CH == 0
            for ic in range(NPG // CH):
                s0 = ic * CH
                o1T = wk.tile([2, CH], f32, tag="o1T")
                nc.sync.dma_start(
                    out=o1T[:],
                    in_=o1f[s0:s0 + CH, :].rearrange("r c -> c r"))
                dv3 = wk.tile([3, CH], f32, tag="dv3")
                for c in range(3):
                    nc.sync.dma_start(
                        out=dv3[c:c + 1, :],
                        in_=dvf[s0:s0 + CH, :].rearrange("r one -> one r"))
                y2t = wk.tile([3, CH], f32, tag="y2t")
                for it in range(CH // 512):
                    s = it * 512
                    p1 = ps.tile([64, 512], f32, tag="p1")
                    nc.tensor.matmul(out=p1[:], lhsT=W1sb[:],
                                     rhs=o1T[:, s:s + 512],
                                     start=True, stop=True)
                    h1 = wk.tile([64, 512], f32, tag="h1")
                    nc.scalar.activation(h1[:], p1[:], Act.Relu,
                                         bias=b1sb[:])
                    p2 = ps.tile([3, 512], f32, tag="p2")
                    nc.tensor.matmul(out=p2[:], lhsT=W2sb[:],
                                     rhs=h1[:], start=True, stop=True)
                    nc.vector.tensor_tensor(out=y2t[:, s:s + 512],
                                            in0=p2[:],
                                            in1=dv3[:, s:s + 512],
                                            op=Alu.mult)
                nc.sync.dma_start(
                    out=y2loc[s0:s0 + CH, :].rearrange("r c -> c r"),
                    in_=y2t[:])

            # ---------- AllGather y2 ----------
            y2tab = dr.tile([NCORES * NPG, 3], f32)
            nc.gpsimd.collective_compute(
                "AllGather",
                Alu.bypass,
                replica_groups=[list(range(NCORES))],
                ins=[y2loc.opt()],
                outs=[y2tab.opt()],
            )

            # y2 of own nodes, grid layout
            y2og = sb.tile([128, NPP * 3], f32)
            nc.sync.dma_start(
                out=y2og[:],
                in_=y2loc[:].rearrange("(p j) c -> p (j c)", p=128))

            # ---------- L2 gather ----------
            g2 = sb.tile([128, NCOLS * 3], f32)
            for i in range(NCOLS):
                nc.gpsimd.indirect_dma_start(
                    out=g2[:, 3 * i:3 * i + 3],
                    out_offset=None,
                    in_=y2tab[:],
                    in_offset=bass.IndirectOffsetOnAxis(
                        ap=offs2_t[:, i:i + 1], axis=0),
                )

            # ---------- L2 class reductions ----------
            agg2 = sb.tile([128, NPP * 3], f32)
            for (c0, a0, R, K) in class_blocks:
                nc.vector.tensor_reduce(
                    out=agg2[:, a0 * 3:(a0 + R) * 3].rearrange(
                        "p (r c) -> p r c", c=3),
                    in_=g2[:, c0 * 3:(c0 + R * K) * 3].rearrange(
                        "p (r k c) -> p r c k", r=R, k=K),
                    axis=Ax.X,
                    op=Alu.add,
                )

            # out2 = (agg2 + pm1 * y2own) * dinv + b2
            u1 = wk.tile([128, NPP * 3], f32, tag="u1")
            nc.vector.tensor_tensor(out=as3(u1[:], 3), in0=as3(y2og[:], 3),
                                    in1=bcast(pm1[:], 3), op=Alu.mult)
            u2 = wk.tile([128, NPP * 3], f32, tag="u2")
            nc.vector.tensor_tensor(out=as3(u2[:], 3), in0=as3(agg2[:], 3),
                                    in1=as3(u1[:], 3), op=Alu.add)
            u3 = wk.tile([128, NPP * 3], f32, tag="u3")
            nc.vector.tensor_tensor(out=as3(u3[:], 3), in0=as3(u2[:], 3),
                                    in1=bcast(dvg[:], 3), op=Alu.mult)
            out2 = sb.tile([128, NPP * 3], f32)
            nc.vector.tensor_tensor(
                out=as3(out2[:], 3), in0=as3(u3[:], 3),
                in1=b2sb[:].unsqueeze(1).to_broadcast([128, NPP, 3]),
                op=Alu.add)

            # ---------- pooling: one-hot over 64 graphs, PE accumulate ----
            gidx = sb.tile([128, NPP], f32)
            nc.sync.dma_start(out=gidx[:], in_=gidx_d.ap())
            iota = sb.tile([128, 64], f32)
            nc.sync.dma_start(out=iota[:], in_=iota_d.ap())
            ppool = ps.tile([64, 3], f32, tag="pp")
            OHC = 8
            assert NPP % OHC == 0
            for jc in range(NPP // OHC):
                j0 = jc * OHC
                oh = wk.tile([128, OHC * 64], f32, tag="oh")
                nc.vector.tensor_tensor(
                    out=oh[:].rearrange("p (j g) -> p j g", g=64),
                    in0=gidx[:, j0:j0 + OHC].unsqueeze(2).to_broadcast(
                        [128, OHC, 64]),
                    in1=iota[:].unsqueeze(1).to_broadcast([128, OHC, 64]),
                    op=Alu.is_equal,
                )
                for j in range(OHC):
                    jj = j0 + j
                    nc.tensor.matmul(out=ppool[:],
                                     lhsT=oh[:, 64 * j:64 * (j + 1)],
                                     rhs=out2[:, 3 * jj:3 * jj + 3],
                                     start=(jj == 0), stop=(jj == NPP - 1))

            cntsb = wk.tile([64, 1], f32, tag="cnt")
            nc.sync.dma_start(out=cntsb[:], in_=cnt_d.ap())
            rcnt = wk.tile([64, 1], f32, tag="rcnt")
            nc.vector.reciprocal(rcnt[:], cntsb[:])
            pooled = wk.tile([64, 3], f32, tag="pool")
            nc.vector.tensor_tensor(out=pooled[:], in0=ppool[:],
                                    in1=rcnt[:].to_broadcast([64, 3]),
                                    op=Alu.mult)
            # log_softmax
            mx = wk.tile([64, 1], f32, tag="mx")
            nc.vector.tensor_reduce(out=mx[:], in_=pooled[:], axis=Ax.X,
                                    op=Alu.max)
            z = wk.tile([64, 3], f32, tag="z")
            nc.vector.tensor_tensor(out=z[:], in0=pooled[:],
                                    in1=mx[:].to_broadcast([64, 3]),
                                    op=Alu.subtract)
            ez = wk.tile([64, 3], f32, tag="ez")
            nc.scalar.activation(ez[:], z[:], Act.Exp)
            sz = wk.tile([64, 1], f32, tag="sz")
            nc.vector.tensor_reduce(out=sz[:], in_=ez[:], axis=Ax.X,
                                    op=Alu.add)
            lz = wk.tile([64, 1], f32, tag="lz")
            nc.scalar.activation(lz[:], sz[:], Act.Ln)
            outz = wk.tile([64, 3], f32, tag="outz")
            nc.vector.tensor_tensor(out=outz[:], in0=z[:],
                                    in1=lz[:].to_broadcast([64, 3]),
                                    op=Alu.subtract)
            nc.sync.dma_start(out=out_d.ap(), in_=outz[:])

    nc.finalize()
    return nc


def _axon_devices_available():
    try:
        import jax

        return any("NC" in str(d) or "axon" in str(d).lower()
                   for d in jax.devices())
    except Exception:
        return False


def _kernel_subprocess(x, edge_index, batch, W1, b1, W2, b2):
    """Re-run in a child process with a clean jax platform (used when the
    caller's process pinned jax to cpu)."""
    import subprocess
    import tempfile

    d = tempfile.mkdtemp()
    inp = os.path.join(d, "in.npz")
    outp = os.path.join(d, "out.npy")
    np.savez(inp, x=x, edge_index=edge_index, batch=batch, W1=W1, b1=b1,
             W2=W2, b2=b2)
    code = (
        "import numpy as np, sys\n"
        f"sys.path.insert(0, {os.path.dirname(os.path.abspath(__file__))!r})\n"
        "import kernel as km\n"
        f"d = np.load({inp!r})\n"
        "out = km.kernel(**{k: d[k] for k in d.files})\n"
        f"np.save({outp!r}, out)\n"
    )
    env = dict(os.environ)
    env.pop("JAX_PLATFORMS", None)
    env["GCN_KERNEL_CHILD"] = "1"
    subprocess.run([sys.executable, "-c", code], check=True, env=env)
    return np.load(outp)


def kernel(x, edge_index, batch, W1, b1, W2, b2):
    global LAST_EXEC_NS, LAST_RESULTS
    if TRACE:
        _install_ntff_shim()
    if (not os.environ.get("GCN_KERNEL_CHILD")
            and not _axon_devices_available()):
        return _kernel_subprocess(x, edge_index, batch, W1, b1, W2, b2)
    from concourse.bass_utils import run_bass_kernel_spmd

    x = np.ascontiguousarray(np.asarray(x, dtype=np.float32))
    src = np.asarray(edge_index[0]).astype(np.int64)
    dst = np.asarray(edge_index[1]).astype(np.int64)
    batch = np.asarray(batch).astype(np.int64)
    W1 = np.asarray(W1, dtype=np.float32)
    b1 = np.asarray(b1, dtype=np.float32)
    W2 = np.asarray(W2, dtype=np.float32)
    b2 = np.asarray(b2, dtype=np.float32)

    st = _host_structure(src, dst, batch)
    NCOLS, NPP, NPG = st["NCOLS"], st["NPP"], st["NPG"]

    # natural-order tables (replicated to all cores)
    x_nat = np.zeros((N1, 2), np.float32)
    x_nat[:N] = x
    deg_nat = np.ones((N1, 1), np.float32)
    deg_nat[:N, 0] = (st["deg"] + 1).astype(np.float32)

    # x in grid order (pure row movement; zero for dummy rows)
    x_g = np.zeros((NCORES * NPG, 2), np.float32)
    x_g[st["gflat_of"]] = x
    x_g = x_g.reshape(NCORES, 128, NPP * 2)

    iota64 = np.broadcast_to(np.arange(64, dtype=np.float32), (128, 64)).copy()

    nc = _build_program(NCOLS, NPP, st["class_blocks"])

    in_maps = []
    for c in range(NCORES):
        in_maps.append({
            "x_nat": x_nat,
            "deg_nat": deg_nat,
            "offs1": st["offs1"][c],
            "offs2": st["offs2"][c],
            "deg_g": st["deg_g"][c],
            "pm1_g": st["pm1_g"][c],
            "gidx_g": st["gidx_g"][c],
            "x_g": x_g[c],
            "W1": W1,
            "b1c": b1.reshape(64, 1),
            "W2": W2,
            "b2r": np.broadcast_to(b2, (128, 3)).copy(),
            "iota64": iota64,
            "cnt": st["cnt"][c].reshape(GPC, 1),
        })

    kwargs = {}
    if TRACE:
        kwargs = dict(trace=True,
                      trace_cores=TRACE_CORES or list(range(NCORES)))
    res = run_bass_kernel_spmd(nc, in_maps, core_ids=list(range(NCORES)),
                               **kwargs)
    LAST_EXEC_NS = res.exec_time_ns
    LAST_RESULTS = res
    out = np.concatenate([res.results[c]["out"] for c in range(NCORES)], axis=0)
    return out.astype(np.float32)
